# revision 1
# baseline (speedup 1.0000x reference)
"""DAGNN forward on 8 Trainium2 NeuronCores.

Reference computation (see problem):
    h = relu(x @ W1 + b1) @ W2 + b2            # dense front
    H_k = A_norm^k h, k=0..10                  # 10 SpMM hops (A from row/col/edge_w)
    S = sigmoid(H @ s); out = sum_k S[:,k] * H_k

Strategy:
 - Shard destination nodes across 8 cores (12500 rows each, padded to 12800 =
   4 quarters x 25 blocks x 128).
 - Dense front: per-core tiled matmuls (bf16 on PE), PE transpose to row layout.
 - Per hop: AllGather h (bf16, duplicated to 256B rows) into a shared DRAM
   table; dma_gather source rows per edge (edges sorted by source-window for
   int16 indices, then by dest block); build one-hot S1 matrices on DVE via
   iota-compare; segment-reduce via PE matmuls accumulating in PSUM per
   128-dest block; evict into SBUF y.
 - Edge weights: reference edge_w is separable (w_ij = dinv_i*dinv_j); fold
   dinv_col into the gathered table and dinv_row into the eviction. Fallback:
   per-edge weight multiply into S1.
 - Self loops are removed from the edge stream and applied directly.
 - Final combine S_k = sigmoid(h_k . s), out += S_k * h_k folded into each hop.

kernel() is self-contained: hardcodes all shapes, builds the plan on the host
with numpy, compiles one SPMD Bass program, runs it on cores 0-7.
"""
import os
import numpy as np
import ml_dtypes

N = 100000
E_IN = 3300000
HOP = 10
IN_DIM, HID_DIM, OUT_DIM = 512, 512, 64

NCORES = 8
P = 128
R_REAL = 12500            # real rows per core
QUART_REAL = 3125         # real rows per quarter
QUART = 3200              # padded quarter (25 blocks)
R_CORE = 4 * QUART        # 12800 rows per core
NBLK = R_CORE // P        # 100 dest blocks per core
GQ = NCORES * QUART       # 25600 global rows per window (< int16 range)
NT = 512                  # row-tile width for dense front
GCH = int(os.environ.get("K_GCH", "4096"))  # edges per gather call
NQUEUES = 4

_bf16 = ml_dtypes.bfloat16


def _build_plan(row, col, edge_w):
    """Host-side graph preprocessing. Returns per-core streams + static plan.

    Node v (original): core c = v // R_REAL, l = v % R_REAL,
    quarter q = l // QUART_REAL, i = l % QUART_REAL.
    y position p_loc = q*QUART + i. Gather id within window q:
    gid = c*QUART + i  (window q table = AllGather of all cores' quarter q).
    """
    row = np.asarray(row).astype(np.int64)
    col = np.asarray(col).astype(np.int64)
    edge_w = np.asarray(edge_w).astype(np.float64)

    c_r, l_r = row // R_REAL, row % R_REAL
    q_r, i_r = l_r // QUART_REAL, l_r % QUART_REAL
    ploc_r = q_r * QUART + i_r
    c_c, l_c = col // R_REAL, col % R_REAL
    q_c, i_c = l_c // QUART_REAL, l_c % QUART_REAL
    gid_c = c_c * QUART + i_c          # index within window q_c

    selfmask = row == col
    # per-node self weight (sum over duplicate self edges)
    wself = np.zeros(N, np.float64)
    np.add.at(wself, row[selfmask], edge_w[selfmask])

    # separability check: dinv from self loops (min over duplicates: each
    # self-edge instance carries dinv_i^2, the sum may double-count)
    wmin = np.full(N, np.inf)
    np.minimum.at(wmin, row[selfmask], edge_w[selfmask])
    dinv = np.sqrt(np.maximum(np.where(np.isfinite(wmin), wmin, 0.0), 0.0))
    sep = False
    nz = ~selfmask
    if np.all(dinv > 0):
        approx = dinv[row[nz]] * dinv[col[nz]]
        err = np.abs(approx - edge_w[nz])
        rel = err / np.maximum(np.abs(edge_w[nz]), 1e-30)
        sep = rel.max() < 1e-4
    if sep:
        dinv_r = dinv
        dinv_c = dinv
    else:
        dinv_r = np.ones(N)
        dinv_c = np.ones(N)

    # non-self edges, grouped per (dest core, src window, dest block)
    er, ec, ew = row[nz], col[nz], edge_w[nz]
    e_core = c_r[nz]
    e_win = q_c[nz]
    e_gid = gid_c[nz]
    e_ploc = ploc_r[nz]
    e_blk = e_ploc // P
    e_rel = e_ploc % P
    e_wval = ew / (dinv_r[er] * dinv_c[ec]) if sep else ew

    # sort edges by (core, win, blk) -> cells; stable order inside
    order = np.lexsort((e_blk, e_win, e_core))
    e_core = e_core[order]; e_win = e_win[order]; e_blk = e_blk[order]
    e_gid = e_gid[order]; e_rel = e_rel[order]; e_wval = e_wval[order]

    # cell counts [core, win, blk]
    cell_id = (e_core * 4 + e_win) * NBLK + e_blk
    counts = np.bincount(cell_id, minlength=NCORES * 4 * NBLK).reshape(
        NCORES, 4, NBLK)
    # equalize across cores, round up to multiple of P
    cell_pad = counts.max(axis=0)
    cell_pad = ((cell_pad + P - 1) // P) * P          # [4, NBLK]
    n_edges_pad = int(cell_pad.sum())                 # same for every core

    # build padded per-core streams
    idx_stream = np.zeros((NCORES, n_edges_pad), np.int16)
    rel_stream = np.full((NCORES, n_edges_pad), -1.0, np.float64)
    w_stream = np.zeros((NCORES, n_edges_pad), np.float64)

    # cell start offsets in the padded stream (win-major, then block)
    cell_starts = np.zeros((4, NBLK), np.int64)
    off = 0
    for w in range(4):
        for b in range(NBLK):
            cell_starts[w, b] = off
            off += cell_pad[w, b]
    assert off == n_edges_pad

    # per-(core,cell) insertion: compute positions vectorized
    # rank of each edge within its (core,win,blk) cell
    # edges already sorted by cell; rank = index - first_index_of_cell
    first_idx = np.zeros(NCORES * 4 * NBLK, np.int64)
    cid_sorted = (e_core * 4 + e_win) * NBLK + e_blk
    starts_in_sorted = np.searchsorted(cid_sorted, np.arange(NCORES * 4 * NBLK))
    first_idx = starts_in_sorted
    rank = np.arange(len(cid_sorted)) - first_idx[cid_sorted]
    pos = cell_starts[e_win, e_blk] + rank
    idx_stream[e_core, pos] = e_gid.astype(np.int16)
    rel_stream[e_core, pos] = e_rel
    w_stream[e_core, pos] = e_wval

    # gather calls: per window, chunks of up to GCH edges
    win_starts = [int(cell_starts[w, 0]) for w in range(4)] + [n_edges_pad]
    calls = []          # (win, start, n_edges)
    for w in range(4):
        s, e = win_starts[w], win_starts[w + 1]
        while s < e:
            n = min(GCH, e - s)
            calls.append((w, s, n))
            s += n

    # matmul chunk -> cell map: chunk t covers edges [t*P,(t+1)*P)
    n_mm = n_edges_pad // P
    chunk_blk = np.zeros(n_mm, np.int64)
    chunk_win = np.zeros(n_mm, np.int64)
    for w in range(4):
        for b in range(NBLK):
            s = cell_starts[w, b]
            n = cell_pad[w, b]
            chunk_blk[s // P:(s + n) // P] = b
            chunk_win[s // P:(s + n) // P] = w

    plan = {
        "sep": sep,
        "n_edges_pad": n_edges_pad,
        "n_mm": n_mm,
        "calls": calls,
        "chunk_blk": chunk_blk,
        "chunk_win": chunk_win,
        "cell_pad": cell_pad,
        "cell_starts": cell_starts,
    }

    # per-node vectors in [p, block] layout per core
    def to_blocks(vec_by_node):
        out = np.zeros((NCORES, P, NBLK), np.float32)
        v = np.asarray(vec_by_node, np.float64)
        cores = np.arange(N) // R_REAL
        l = np.arange(N) % R_REAL
        q, i = l // QUART_REAL, l % QUART_REAL
        ploc = q * QUART + i
        out[cores, ploc % P, ploc // P] = v
        return out

    vecs = {
        "dinv_r": to_blocks(dinv_r),
        "dinv_c": to_blocks(dinv_c),
        "wself": to_blocks(wself),
    }
    return plan, idx_stream, rel_stream, w_stream, vecs


def _pack_idx16(idx_stream):
    """[NCORES, L*16] -> [NCORES, 128, L] int16, wrapped in 16 partitions and
    replicated to all 8 Q7 core groups."""
    nc_, n = idx_stream.shape
    L = n // 16
    out = np.zeros((nc_, P, L), np.int16)
    w = idx_stream.reshape(nc_, L, 16)
    for rep in range(8):
        out[:, rep * 16:(rep + 1) * 16, :] = np.swapaxes(w, 1, 2)
    return out


def _perm_x(x):
    """x [N, IN] -> per-core padded, y-position order, transposed bf16."""
    xt = np.zeros((NCORES, IN_DIM, R_CORE), np.float32)
    v = np.arange(N)
    cores = v // R_REAL
    l = v % R_REAL
    ploc = (l // QUART_REAL) * QUART + (l % QUART_REAL)
    xf = np.asarray(x, np.float32)
    for c in range(NCORES):
        m = cores == c
        xt[c][:, ploc[m]] = xf[m].T
    return xt.astype(_bf16)


def _unperm_out(res):
    """per-core [R_CORE, OUT] -> [N, OUT] original order."""
    out = np.zeros((N, OUT_DIM), np.float32)
    v = np.arange(N)
    cores = v // R_REAL
    l = v % R_REAL
    ploc = (l // QUART_REAL) * QUART + (l % QUART_REAL)
    for c in range(NCORES):
        m = cores == c
        out[m] = res[c][ploc[m]]
    return out


def _build_program(plan, nonsep):
    import concourse.bacc as bacc
    import concourse.mybir as mybir
    from concourse.tile import TileContext

    dt = mybir.dt
    nc = bacc.Bacc(num_devices=NCORES, num_swdge_queues=int(os.environ.get("K_NQ", str(NQUEUES))),
                   dynamic_dma_scratch_size=int(os.environ.get("K_SCRATCH", "16384")))
    n_hops = int(os.environ.get("K_HOPS", str(HOP)))
    skip_ag = os.environ.get("K_SKIP_AG", "0") == "1"
    skip_gather = os.environ.get("K_SKIP_GATHER", "0") == "1"

    n_mm = plan["n_mm"]
    n_edges_pad = plan["n_edges_pad"]
    calls = plan["calls"]
    chunk_blk = plan["chunk_blk"]
    chunk_win = plan["chunk_win"]
    cell_pad = plan["cell_pad"]
    first_win = {}
    for b in range(NBLK):
        for w in range(4):
            if cell_pad[w, b] > 0:
                first_win[b] = w
                break

    # ---- inputs
    xT = nc.dram_tensor("xT", [IN_DIM, R_CORE], dt.bfloat16, kind="ExternalInput")
    W1b = nc.dram_tensor("W1b", [IN_DIM, HID_DIM], dt.bfloat16, kind="ExternalInput")
    W2b = nc.dram_tensor("W2b", [HID_DIM, OUT_DIM], dt.bfloat16, kind="ExternalInput")
    b1c = nc.dram_tensor("b1c", [P, HID_DIM // P], dt.float32, kind="ExternalInput")
    b2c = nc.dram_tensor("b2c", [OUT_DIM, 1], dt.float32, kind="ExternalInput")
    ident = nc.dram_tensor("ident", [P, P], dt.float32, kind="ExternalInput")
    iota_in = nc.dram_tensor("iota_in", [P, P], dt.bfloat16, kind="ExternalInput")
    s_rep = nc.dram_tensor("s_rep", [P, OUT_DIM], dt.float32, kind="ExternalInput")
    dinv_r_t = nc.dram_tensor("dinv_r_t", [P, NBLK], dt.float32, kind="ExternalInput")
    dinv_c_t = nc.dram_tensor("dinv_c_t", [P, NBLK], dt.float32, kind="ExternalInput")
    wself_t = nc.dram_tensor("wself_t", [P, NBLK], dt.float32, kind="ExternalInput")
    idx16 = nc.dram_tensor("idx16", [P, n_edges_pad // 16], dt.int16,
                           kind="ExternalInput")
    rel_in = nc.dram_tensor("rel_in", [P, n_mm], dt.bfloat16, kind="ExternalInput")
    if nonsep:
        w_in = nc.dram_tensor("w_in", [P, n_mm], dt.bfloat16, kind="ExternalInput")
    out_d = nc.dram_tensor("out", [R_CORE, OUT_DIM], dt.float32,
                           kind="ExternalOutput")

    # ---- internal DRAM for collectives
    cc_in = nc.dram_tensor("cc_in", [R_CORE, P], dt.bfloat16, kind="Internal")
    cc_out = nc.dram_tensor("cc_out", [4 * GQ, P], dt.bfloat16, kind="Internal",
                            addr_space="Shared")
    rg = [list(range(NCORES))]

    KB = IN_DIM // P     # 4 k-blocks
    MB = HID_DIM // P    # 4 m-blocks
    NRT_ = R_CORE // NT  # 25 row tiles

    with TileContext(nc) as tc:
        with (
            tc.tile_pool(name="resid", bufs=1) as resid,
            tc.tile_pool(name="sbuf", bufs=3) as sbuf,
            tc.tile_pool(name="one", bufs=1) as onep,
            tc.tile_pool(name="gpool", bufs=2) as gpool,
            tc.tile_pool(name="s1pool", bufs=2) as s1pool,
            tc.tile_pool(name="idxp", bufs=2) as idxp,
            tc.tile_pool(name="psum", bufs=1, space="PSUM") as psum,
        ):
            # resident constants / state
            iota_t = resid.tile([P, P], dt.bfloat16)
            nc.sync.dma_start(iota_t[:], iota_in[:])
            ident_t = resid.tile([P, P], dt.float32)
            nc.sync.dma_start(ident_t[:], ident[:])
            s_t = resid.tile([P, OUT_DIM], dt.float32)
            nc.sync.dma_start(s_t[:], s_rep[:])
            dinvr = resid.tile([P, NBLK], dt.float32)
            nc.sync.dma_start(dinvr[:], dinv_r_t[:])
            dinvc = resid.tile([P, NBLK], dt.float32)
            nc.sync.dma_start(dinvc[:], dinv_c_t[:])
            wself = resid.tile([P, NBLK], dt.float32)
            nc.sync.dma_start(wself[:], wself_t[:])
            rel_t = resid.tile([P, n_mm], dt.bfloat16)
            nc.sync.dma_start(rel_t[:], rel_in[:])
            if nonsep:
                w_t = resid.tile([P, n_mm], dt.bfloat16)
                nc.sync.dma_start(w_t[:], w_in[:])

            w1_t = resid.tile([P, KB, HID_DIM], dt.bfloat16)
            nc.sync.dma_start(
                w1_t[:], W1b[:].rearrange("(kb p) m -> p kb m", p=P))
            w2_t = resid.tile([P, KB, OUT_DIM], dt.bfloat16)
            nc.sync.dma_start(
                w2_t[:], W2b[:].rearrange("(kb p) m -> p kb m", p=P))
            b1_t = resid.tile([P, MB], dt.float32)
            nc.sync.dma_start(b1_t[:], b1c[:])
            b2_t = resid.tile([OUT_DIM, 1], dt.float32)
            nc.sync.dma_start(b2_t[:], b2c[:])

            # state buffers [P, NBLK, OUT]
            h_prev = resid.tile([P, NBLK, OUT_DIM], dt.float32)
            y_cur = resid.tile([P, NBLK, OUT_DIM], dt.float32)
            out_acc = resid.tile([P, NBLK, OUT_DIM], dt.float32)
            nc.vector.memset(out_acc[:], 0.0)
            sk_t = resid.tile([P, NBLK], dt.float32)
            nc.vector.memset(y_cur[:], 0.0)

            # ---------------- dense front: h0 -> h_prev ----------------
            for rt in range(NRT_):
                xk = []
                for kb in range(KB):
                    t = sbuf.tile([P, NT], dt.bfloat16, tag="xk", bufs=8, name="xkt")
                    nc.sync.dma_start(
                        t[:], xT[kb * P:(kb + 1) * P, rt * NT:(rt + 1) * NT])
                    xk.append(t)
                h0 = []
                for mb in range(MB):
                    acc = psum.tile([P, NT], dt.float32, space="PSUM", tag="mm1", bufs=1)
                    for kb in range(KB):
                        nc.tensor.matmul(
                            out=acc[:],
                            lhsT=w1_t[:, kb, mb * P:(mb + 1) * P],
                            rhs=xk[kb][:],
                            start=(kb == 0), stop=(kb == KB - 1))
                    h0t = sbuf.tile([P, NT], dt.bfloat16, tag="h0", bufs=8)
                    nc.scalar.activation(
                        h0t[:], acc[:], mybir.ActivationFunctionType.Relu,
                        bias=b1_t[:, mb:mb + 1], scale=1.0)
                    h0.append(h0t)
                acc2 = psum.tile([P, NT], dt.float32, space="PSUM", tag="mm2", bufs=1)
                for kb in range(KB):
                    nc.tensor.matmul(
                        out=acc2[:OUT_DIM, :],
                        lhsT=w2_t[:, kb, :],
                        rhs=h0[kb][:],
                        start=(kb == 0), stop=(kb == KB - 1))
                hT = sbuf.tile([OUT_DIM, NT], dt.float32, tag="hT")
                nc.vector.tensor_scalar(
                    out=hT[:], in0=acc2[:OUT_DIM, :],
                    scalar1=b2_t[:, 0:1], scalar2=None,
                    op0=mybir.AluOpType.add)
                # transpose [64, NT] -> row layout [P, 64] x (NT/P)
                for st in range(NT // P):
                    pt = psum.tile([P, OUT_DIM], dt.float32, space="PSUM",
                                   tag="tr", bufs=2)
                    nc.tensor.transpose(
                        out=pt[:, :],
                        in_=hT[:, st * P:(st + 1) * P],
                        identity=ident_t[:OUT_DIM, :OUT_DIM])
                    blk = rt * (NT // P) + st
                    nc.vector.tensor_copy(h_prev[:, blk, :], pt[:, :])

            # ---------------- hops ----------------
            for hop in range(n_hops):
                # epilogue of previous state: S_k & out_acc for h_prev
                # S_k = sigmoid(sum_f h_prev*s) ; out_acc += S_k * h_prev
                tmp = onep.tile([P, NBLK, OUT_DIM], dt.float32, tag="tmp")
                nc.vector.tensor_tensor(
                    out=tmp[:], in0=h_prev[:],
                    in1=s_t[:].unsqueeze(1).to_broadcast([P, NBLK, OUT_DIM]),
                    op=mybir.AluOpType.mult)
                nc.vector.tensor_reduce(
                    out=sk_t[:], in_=tmp[:], axis=mybir.AxisListType.X,
                    op=mybir.AluOpType.add)
                nc.scalar.activation(
                    sk_t[:], sk_t[:], mybir.ActivationFunctionType.Sigmoid)
                nc.vector.tensor_tensor(
                    out=tmp[:], in0=h_prev[:],
                    in1=sk_t[:].unsqueeze(2).to_broadcast([P, NBLK, OUT_DIM]),
                    op=mybir.AluOpType.mult)
                nc.vector.tensor_add(out_acc[:], out_acc[:], tmp[:])

                # h_dup staging: hd = dinv_c * h_prev, cast bf16, dup halves
                hd = onep.tile([P, NBLK, OUT_DIM], dt.bfloat16, tag="hd")
                nc.vector.tensor_tensor(
                    out=hd[:], in0=h_prev[:],
                    in1=dinvc[:].unsqueeze(2).to_broadcast([P, NBLK, OUT_DIM]),
                    op=mybir.AluOpType.mult)
                # write both 64-halves of cc_in rows
                cc_v = cc_in[:].rearrange("(b p) f -> p b f", p=P)
                nc.sync.dma_start(cc_v[:, :, 0:OUT_DIM], hd[:])
                nc.sync.dma_start(cc_v[:, :, OUT_DIM:2 * OUT_DIM], hd[:])

                for q in range(4 if not skip_ag else 0):
                    nc.gpsimd.collective_compute(
                        "AllGather", mybir.AluOpType.bypass,
                        ins=[cc_in[q * QUART:(q + 1) * QUART, :]],
                        outs=[cc_out[q * GQ:(q + 1) * GQ, :]],
                        replica_groups=rg)

                # gather + S1 + PE reduce
                open_blk = {}
                ci = 0  # global chunk index
                for (w, start, n_e) in calls:
                    ncall = n_e // P
                    idxt = idxp.tile([P, GCH // 16], dt.int16, tag="idx")
                    nc.sync.dma_start(
                        idxt[:, :n_e // 16],
                        idx16[:, start // 16:(start + n_e) // 16])
                    g = gpool.tile([P, GCH // P, P], dt.bfloat16, tag="g")
                    if skip_gather:
                        nc.vector.memset(g[:, :ncall, :], 0.5)
                    else:
                        nc.gpsimd.dma_gather(
                            out_ap=g[:, :ncall, :],
                            in_ap=cc_out[w * GQ:(w + 1) * GQ, :],
                            idxs_ap=idxt[:, :n_e // 16],
                            num_idxs=n_e,
                            num_idxs_reg=n_e,
                            elem_size=P,
                            single_packet=os.environ.get("K_SP", "0") == "1",
                            queue_num=ci % int(os.environ.get("K_NQ", str(NQUEUES))),
                        )
                    s1 = s1pool.tile([P, GCH // P, P], dt.bfloat16, tag="s1")
                    nc.vector.tensor_tensor(
                        out=s1[:, :ncall, :],
                        in0=iota_t[:].unsqueeze(1).to_broadcast([P, ncall, P]),
                        in1=rel_t[:, ci:ci + ncall].unsqueeze(2)
                            .to_broadcast([P, ncall, P]),
                        op=mybir.AluOpType.is_equal)
                    if nonsep:
                        nc.vector.tensor_tensor(
                            out=s1[:, :ncall, :], in0=s1[:, :ncall, :],
                            in1=w_t[:, ci:ci + ncall].unsqueeze(2)
                                .to_broadcast([P, ncall, P]),
                            op=mybir.AluOpType.mult)
                    for j in range(ncall):
                        t = ci + j
                        b = int(chunk_blk[t])
                        wv = int(chunk_win[t])
                        key = (wv, b)
                        if key not in open_blk:
                            open_blk[key] = psum.tile(
                                [P, OUT_DIM], dt.float32, space="PSUM",
                                tag="acc", name="accb", bufs=4)
                        first = (t == 0 or int(chunk_blk[t - 1]) != b
                                 or int(chunk_win[t - 1]) != wv)
                        last = (t == n_mm - 1 or int(chunk_blk[t + 1]) != b
                                or int(chunk_win[t + 1]) != wv)
                        nc.tensor.matmul(
                            out=open_blk[key][:],
                            lhsT=s1[:, j, :],
                            rhs=g[:, j, 0:OUT_DIM],
                            start=first, stop=last)
                        if last:
                            pt = open_blk.pop(key)
                            if wv == first_win.get(b, -1):
                                nc.vector.tensor_copy(y_cur[:, b, :], pt[:])
                            else:
                                nc.vector.tensor_add(
                                    y_cur[:, b, :], y_cur[:, b, :], pt[:])
                    ci += ncall

                # finalize y: y = dinv_r*y + wself*h_prev
                nc.vector.tensor_tensor(
                    out=y_cur[:], in0=y_cur[:],
                    in1=dinvr[:].unsqueeze(2).to_broadcast([P, NBLK, OUT_DIM]),
                    op=mybir.AluOpType.mult)
                tmp2 = onep.tile([P, NBLK, OUT_DIM], dt.float32, tag="tmp")
                nc.vector.tensor_tensor(
                    out=tmp2[:], in0=h_prev[:],
                    in1=wself[:].unsqueeze(2).to_broadcast([P, NBLK, OUT_DIM]),
                    op=mybir.AluOpType.mult)
                nc.vector.tensor_add(y_cur[:], y_cur[:], tmp2[:])
                # swap: h_prev <- y_cur
                nc.vector.tensor_copy(h_prev[:], y_cur[:])

            # final hop's S_k/out_acc
            tmp = onep.tile([P, NBLK, OUT_DIM], dt.float32, tag="tmp")
            nc.vector.tensor_tensor(
                out=tmp[:], in0=h_prev[:],
                in1=s_t[:].unsqueeze(1).to_broadcast([P, NBLK, OUT_DIM]),
                op=mybir.AluOpType.mult)
            nc.vector.tensor_reduce(
                out=sk_t[:], in_=tmp[:], axis=mybir.AxisListType.X,
                op=mybir.AluOpType.add)
            nc.scalar.activation(
                sk_t[:], sk_t[:], mybir.ActivationFunctionType.Sigmoid)
            nc.vector.tensor_tensor(
                out=tmp[:], in0=h_prev[:],
                in1=sk_t[:].unsqueeze(2).to_broadcast([P, NBLK, OUT_DIM]),
                op=mybir.AluOpType.mult)
            nc.vector.tensor_add(out_acc[:], out_acc[:], tmp[:])

            nc.sync.dma_start(
                out_d[:].rearrange("(b p) f -> p b f", p=P), out_acc[:])

    nc.compile()
    return nc


class _Runner:
    """Compile once via bass2jax/PJRT, execute on demand (timeable)."""

    def __init__(self, nc, in_maps, n_cores=NCORES):
        import jax
        from jax.experimental.shard_map import shard_map
        from jax.sharding import Mesh, PartitionSpec
        import concourse.mybir as mybir
        from concourse import bass2jax

        bass2jax.install_neuronx_cc_hook()
        self._jax = jax
        partition_name = (
            nc.partition_id_tensor.name if nc.partition_id_tensor else None)
        in_names, out_names, out_avals, zero_outs = [], [], [], []
        for alloc in nc.m.functions[0].allocations:
            if not isinstance(alloc, mybir.MemoryLocationSet):
                continue
            name = alloc.memorylocations[0].name
            if alloc.kind == "ExternalInput":
                if name != partition_name and name != (
                        nc.dbg_addr.name if nc.dbg_addr else None):
                    in_names.append(name)
            elif alloc.kind == "ExternalOutput":
                shape = tuple(alloc.tensor_shape)
                dtype = mybir.dt.np(alloc.dtype)
                out_names.append(name)
                out_avals.append(jax.core.ShapedArray(shape, dtype))
                zero_outs.append(np.zeros(shape, dtype))
        n_params = len(in_names)
        n_outs = len(out_avals)
        all_in_names = list(in_names) + list(out_names)
        if nc.dbg_addr is not None:
            all_in_names.append(nc.dbg_addr.name)
        if partition_name is not None:
            all_in_names.append(partition_name)
        donate = tuple(range(n_params, n_params + n_outs))

        def _body(*args):
            operands = list(args)
            if nc.dbg_addr is not None:
                operands.append(jax.numpy.zeros((1, 2), jax.numpy.uint32))
            if partition_name is not None:
                operands.append(bass2jax.partition_id_tensor())
            outs = bass2jax._bass_exec_p.bind(
                *operands,
                out_avals=tuple(out_avals),
                in_names=tuple(all_in_names),
                out_names=tuple(out_names),
                lowering_input_output_aliases=(),
                sim_require_finite=False,
                sim_require_nnan=False,
                nc=nc,
            )
            return tuple(outs)

        devices = jax.devices()[:n_cores]
        mesh = Mesh(np.asarray(devices), ("core",))
        self._fn = jax.jit(
            shard_map(_body, mesh=mesh,
                      in_specs=(PartitionSpec("core"),) * (n_params + n_outs),
                      out_specs=(PartitionSpec("core"),) * len(out_names),
                      check_rep=False),
            donate_argnums=donate, keep_unused=True)
        concat_in = [
            np.concatenate([np.asarray(in_maps[c][k]) for c in range(n_cores)],
                           axis=0)
            for k in in_names
        ]
        self._dev_in = [jax.device_put(a) for a in concat_in]
        jax.block_until_ready(self._dev_in)
        self._zero_outs = [
            np.zeros((n_cores * z.shape[0], *z.shape[1:]), z.dtype)
            for z in zero_outs
        ]
        self.out_names = out_names
        self.out_avals = out_avals
        self.n_cores = n_cores

    def run_once(self):
        outs = self._fn(*self._dev_in, *self._zero_outs)
        self._jax.block_until_ready(outs)
        return outs

    def results(self, outs):
        return [
            {name: np.asarray(outs[i]).reshape(
                self.n_cores, *self.out_avals[i].shape)[c]
             for i, name in enumerate(self.out_names)}
            for c in range(self.n_cores)
        ]

    def time(self, iters=5, warmup=1):
        import time as _time
        for _ in range(warmup):
            self.run_once()
        ts = []
        for _ in range(iters):
            t0 = _time.perf_counter()
            self.run_once()
            ts.append(_time.perf_counter() - t0)
        return min(ts), None


LAST_RUNNER = None


def kernel(x, row, col, edge_w, W1, b1, W2, b2, s):
    global LAST_RUNNER
    plan, idx_stream, rel_stream, w_stream, vecs = _build_plan(row, col, edge_w)
    nonsep = not plan["sep"]

    nc = _build_program(plan, nonsep)

    idx16 = _pack_idx16(idx_stream)
    xt = _perm_x(x)
    n_mm = plan["n_mm"]

    iota = np.broadcast_to(
        np.arange(P, dtype=np.float32), (P, P)).astype(_bf16)
    ident = np.eye(P, dtype=np.float32)
    s_rep = np.broadcast_to(
        np.asarray(s, np.float32).reshape(1, OUT_DIM), (P, OUT_DIM)).copy()
    W1b = np.asarray(W1, np.float32).astype(_bf16)
    W2b = np.asarray(W2, np.float32).astype(_bf16)
    b1c = np.asarray(b1, np.float32).reshape(HID_DIM // P, P).T.copy()
    b2c = np.asarray(b2, np.float32).reshape(OUT_DIM, 1)

    in_maps = []
    for c in range(NCORES):
        m = {
            "xT": xt[c],
            "W1b": W1b, "W2b": W2b, "b1c": b1c, "b2c": b2c,
            "ident": ident, "iota_in": iota, "s_rep": s_rep,
            "dinv_r_t": vecs["dinv_r"][c], "dinv_c_t": vecs["dinv_c"][c],
            "wself_t": vecs["wself"][c],
            "idx16": idx16[c],
            "rel_in": rel_stream[c].reshape(n_mm, P).T.astype(_bf16),
        }
        if nonsep:
            m["w_in"] = w_stream[c].reshape(n_mm, P).T.astype(_bf16)
        in_maps.append(m)

    runner = _Runner(nc, in_maps, NCORES)
    LAST_RUNNER = runner
    outs = runner.run_once()
    res = [m["out"] for m in runner.results(outs)]
    return _unperm_out(res).astype(np.float32)



# revision 3
# speedup vs baseline: 5.0460x; 5.0460x over previous
"""DAGNN forward on 8 Trainium2 NeuronCores.

Reference computation (see problem):
    h = relu(x @ W1 + b1) @ W2 + b2            # dense front
    H_k = A_norm^k h, k=0..10                  # 10 SpMM hops (A from row/col/edge_w)
    S = sigmoid(H @ s); out = sum_k S[:,k] * H_k

Strategy:
 - Shard destination nodes across 8 cores (12500 rows each, padded to 12800 =
   4 quarters x 25 blocks x 128).
 - Dense front: per-core tiled matmuls (bf16 on PE), PE transpose to row layout.
 - Per hop: AllGather h (bf16, duplicated to 256B rows) into a shared DRAM
   table; dma_gather source rows per edge (edges sorted by source-window for
   int16 indices, then by dest block); build one-hot S1 matrices on DVE via
   iota-compare; segment-reduce via PE matmuls accumulating in PSUM per
   128-dest block; evict into SBUF y.
 - Edge weights: reference edge_w is separable (w_ij = dinv_i*dinv_j); fold
   dinv_col into the gathered table and dinv_row into the eviction. Fallback:
   per-edge weight multiply into S1.
 - Self loops are removed from the edge stream and applied directly.
 - Final combine S_k = sigmoid(h_k . s), out += S_k * h_k folded into each hop.

kernel() is self-contained: hardcodes all shapes, builds the plan on the host
with numpy, compiles one SPMD Bass program, runs it on cores 0-7.
"""
import os
import numpy as np
import ml_dtypes

N = 100000
E_IN = 3300000
HOP = 10
IN_DIM, HID_DIM, OUT_DIM = 512, 512, 64

NCORES = 8
P = 128
R_REAL = 12500            # real rows per core
QUART_REAL = 3125         # real rows per quarter
QUART = 3200              # padded quarter (25 blocks)
R_CORE = 4 * QUART        # 12800 rows per core
NBLK = R_CORE // P        # 100 dest blocks per core
GQ = NCORES * QUART       # 25600 global rows per window (< int16 range)
NT = 512                  # row-tile width for dense front
GCH = int(os.environ.get("K_GCH", "4096"))  # edges per gather call
NQUEUES = 4

_bf16 = ml_dtypes.bfloat16


def _build_plan(row, col, edge_w):
    """Host-side graph preprocessing. Returns per-core streams + static plan.

    Node v (original): core c = v // R_REAL, l = v % R_REAL,
    quarter q = l // QUART_REAL, i = l % QUART_REAL.
    y position p_loc = q*QUART + i. Gather id within window q:
    gid = c*QUART + i  (window q table = AllGather of all cores' quarter q).
    """
    row = np.asarray(row).astype(np.int64)
    col = np.asarray(col).astype(np.int64)
    edge_w = np.asarray(edge_w).astype(np.float64)

    c_r, l_r = row // R_REAL, row % R_REAL
    q_r, i_r = l_r // QUART_REAL, l_r % QUART_REAL
    ploc_r = q_r * QUART + i_r
    c_c, l_c = col // R_REAL, col % R_REAL
    q_c, i_c = l_c // QUART_REAL, l_c % QUART_REAL
    gid_c = c_c * QUART + i_c          # index within window q_c

    selfmask = row == col
    # per-node self weight (sum over duplicate self edges)
    wself = np.zeros(N, np.float64)
    np.add.at(wself, row[selfmask], edge_w[selfmask])

    # separability check: dinv from self loops (min over duplicates: each
    # self-edge instance carries dinv_i^2, the sum may double-count)
    wmin = np.full(N, np.inf)
    np.minimum.at(wmin, row[selfmask], edge_w[selfmask])
    dinv = np.sqrt(np.maximum(np.where(np.isfinite(wmin), wmin, 0.0), 0.0))
    sep = False
    nz = ~selfmask
    if np.all(dinv > 0):
        approx = dinv[row[nz]] * dinv[col[nz]]
        err = np.abs(approx - edge_w[nz])
        rel = err / np.maximum(np.abs(edge_w[nz]), 1e-30)
        sep = rel.max() < 1e-4
    if sep:
        dinv_r = dinv
        dinv_c = dinv
    else:
        dinv_r = np.ones(N)
        dinv_c = np.ones(N)

    # non-self edges, grouped per (dest core, src window, dest block)
    er, ec, ew = row[nz], col[nz], edge_w[nz]
    e_core = c_r[nz]
    e_win = q_c[nz]
    e_gid = gid_c[nz]
    e_ploc = ploc_r[nz]
    e_blk = e_ploc // P
    e_rel = e_ploc % P
    e_wval = ew / (dinv_r[er] * dinv_c[ec]) if sep else ew

    # sort edges by (core, win, blk) -> cells; stable order inside
    order = np.lexsort((e_blk, e_win, e_core))
    e_core = e_core[order]; e_win = e_win[order]; e_blk = e_blk[order]
    e_gid = e_gid[order]; e_rel = e_rel[order]; e_wval = e_wval[order]

    # cell counts [core, win, blk]
    cell_id = (e_core * 4 + e_win) * NBLK + e_blk
    counts = np.bincount(cell_id, minlength=NCORES * 4 * NBLK).reshape(
        NCORES, 4, NBLK)
    # equalize across cores, round up to multiple of P
    cell_pad = counts.max(axis=0)
    cell_pad = ((cell_pad + P - 1) // P) * P          # [4, NBLK]
    n_edges_pad = int(cell_pad.sum())                 # same for every core

    # build padded per-core streams
    idx_stream = np.zeros((NCORES, n_edges_pad), np.int16)
    rel_stream = np.full((NCORES, n_edges_pad), -1.0, np.float64)
    w_stream = np.zeros((NCORES, n_edges_pad), np.float64)

    # cell start offsets in the padded stream (win-major, then block)
    cell_starts = np.zeros((4, NBLK), np.int64)
    off = 0
    for w in range(4):
        for b in range(NBLK):
            cell_starts[w, b] = off
            off += cell_pad[w, b]
    assert off == n_edges_pad

    # per-(core,cell) insertion: compute positions vectorized
    # rank of each edge within its (core,win,blk) cell
    # edges already sorted by cell; rank = index - first_index_of_cell
    first_idx = np.zeros(NCORES * 4 * NBLK, np.int64)
    cid_sorted = (e_core * 4 + e_win) * NBLK + e_blk
    starts_in_sorted = np.searchsorted(cid_sorted, np.arange(NCORES * 4 * NBLK))
    first_idx = starts_in_sorted
    rank = np.arange(len(cid_sorted)) - first_idx[cid_sorted]
    pos = cell_starts[e_win, e_blk] + rank
    idx_stream[e_core, pos] = e_gid.astype(np.int16)
    rel_stream[e_core, pos] = e_rel
    w_stream[e_core, pos] = e_wval

    # gather calls: per window, chunks of up to GCH edges
    win_starts = [int(cell_starts[w, 0]) for w in range(4)] + [n_edges_pad]
    calls = []          # (win, start, n_edges)
    for w in range(4):
        s, e = win_starts[w], win_starts[w + 1]
        while s < e:
            n = min(GCH, e - s)
            calls.append((w, s, n))
            s += n

    # matmul chunk -> cell map: chunk t covers edges [t*P,(t+1)*P)
    n_mm = n_edges_pad // P
    chunk_blk = np.zeros(n_mm, np.int64)
    chunk_win = np.zeros(n_mm, np.int64)
    for w in range(4):
        for b in range(NBLK):
            s = cell_starts[w, b]
            n = cell_pad[w, b]
            chunk_blk[s // P:(s + n) // P] = b
            chunk_win[s // P:(s + n) // P] = w

    plan = {
        "sep": sep,
        "n_edges_pad": n_edges_pad,
        "n_mm": n_mm,
        "calls": calls,
        "chunk_blk": chunk_blk,
        "chunk_win": chunk_win,
        "cell_pad": cell_pad,
        "cell_starts": cell_starts,
    }

    # per-node vectors in [p, block] layout per core
    def to_blocks(vec_by_node):
        out = np.zeros((NCORES, P, NBLK), np.float32)
        v = np.asarray(vec_by_node, np.float64)
        cores = np.arange(N) // R_REAL
        l = np.arange(N) % R_REAL
        q, i = l // QUART_REAL, l % QUART_REAL
        ploc = q * QUART + i
        out[cores, ploc % P, ploc // P] = v
        return out

    vecs = {
        "dinv_r": to_blocks(dinv_r),
        "dinv_c": to_blocks(dinv_c),
        "wself": to_blocks(wself),
    }
    return plan, idx_stream, rel_stream, w_stream, vecs


def _pack_idx16(idx_stream):
    """[NCORES, L*16] -> [NCORES, 128, L] int16, wrapped in 16 partitions and
    replicated to all 8 Q7 core groups."""
    nc_, n = idx_stream.shape
    L = n // 16
    out = np.zeros((nc_, P, L), np.int16)
    w = idx_stream.reshape(nc_, L, 16)
    for rep in range(8):
        out[:, rep * 16:(rep + 1) * 16, :] = np.swapaxes(w, 1, 2)
    return out


def _perm_x(x):
    """x [N, IN] -> per-core padded, y-position order, transposed bf16."""
    xt = np.zeros((NCORES, IN_DIM, R_CORE), np.float32)
    v = np.arange(N)
    cores = v // R_REAL
    l = v % R_REAL
    ploc = (l // QUART_REAL) * QUART + (l % QUART_REAL)
    xf = np.asarray(x, np.float32)
    for c in range(NCORES):
        m = cores == c
        xt[c][:, ploc[m]] = xf[m].T
    return xt.astype(_bf16)


def _unperm_out(res):
    """per-core [R_CORE, OUT] -> [N, OUT] original order."""
    out = np.zeros((N, OUT_DIM), np.float32)
    v = np.arange(N)
    cores = v // R_REAL
    l = v % R_REAL
    ploc = (l // QUART_REAL) * QUART + (l % QUART_REAL)
    for c in range(NCORES):
        m = cores == c
        out[m] = res[c][ploc[m]]
    return out


def _build_program(plan, nonsep):
    import concourse.bacc as bacc
    import concourse.mybir as mybir
    from concourse.tile import TileContext

    dt = mybir.dt
    nc = bacc.Bacc(num_devices=NCORES, num_swdge_queues=int(os.environ.get("K_NQ", str(NQUEUES))),
                   dynamic_dma_scratch_size=int(os.environ.get("K_SCRATCH", "16384")))
    n_hops = int(os.environ.get("K_HOPS", str(HOP)))
    skip_ag = os.environ.get("K_SKIP_AG", "0") == "1"
    skip_gather = os.environ.get("K_SKIP_GATHER", "0") == "1"

    n_mm = plan["n_mm"]
    n_edges_pad = plan["n_edges_pad"]
    calls = plan["calls"]
    chunk_blk = plan["chunk_blk"]
    chunk_win = plan["chunk_win"]
    cell_pad = plan["cell_pad"]
    first_win = {}
    for b in range(NBLK):
        for w in range(4):
            if cell_pad[w, b] > 0:
                first_win[b] = w
                break

    # ---- inputs
    xT = nc.dram_tensor("xT", [IN_DIM, R_CORE], dt.bfloat16, kind="ExternalInput")
    W1b = nc.dram_tensor("W1b", [IN_DIM, HID_DIM], dt.bfloat16, kind="ExternalInput")
    W2b = nc.dram_tensor("W2b", [HID_DIM, OUT_DIM], dt.bfloat16, kind="ExternalInput")
    b1c = nc.dram_tensor("b1c", [P, HID_DIM // P], dt.float32, kind="ExternalInput")
    b2c = nc.dram_tensor("b2c", [OUT_DIM, 1], dt.float32, kind="ExternalInput")
    ident = nc.dram_tensor("ident", [P, P], dt.float32, kind="ExternalInput")
    iota_in = nc.dram_tensor("iota_in", [P, P], dt.bfloat16, kind="ExternalInput")
    s_rep = nc.dram_tensor("s_rep", [P, OUT_DIM], dt.float32, kind="ExternalInput")
    dinv_r_t = nc.dram_tensor("dinv_r_t", [P, NBLK], dt.float32, kind="ExternalInput")
    dinv_c_t = nc.dram_tensor("dinv_c_t", [P, NBLK], dt.float32, kind="ExternalInput")
    wself_t = nc.dram_tensor("wself_t", [P, NBLK], dt.float32, kind="ExternalInput")
    idx16 = nc.dram_tensor("idx16", [P, n_edges_pad // 16], dt.int16,
                           kind="ExternalInput")
    rel_in = nc.dram_tensor("rel_in", [P, n_mm], dt.bfloat16, kind="ExternalInput")
    if nonsep:
        w_in = nc.dram_tensor("w_in", [P, n_mm], dt.bfloat16, kind="ExternalInput")
    out_d = nc.dram_tensor("out", [R_CORE, OUT_DIM], dt.float32,
                           kind="ExternalOutput")

    # ---- internal DRAM for collectives
    cc_in = nc.dram_tensor("cc_in", [R_CORE, P], dt.bfloat16, kind="Internal")
    cc_out = nc.dram_tensor("cc_out", [4 * GQ, P], dt.bfloat16, kind="Internal",
                            addr_space="Shared")
    rg = [list(range(NCORES))]

    KB = IN_DIM // P     # 4 k-blocks
    MB = HID_DIM // P    # 4 m-blocks
    NRT_ = R_CORE // NT  # 25 row tiles

    with TileContext(nc) as tc:
        with (
            tc.tile_pool(name="resid", bufs=1) as resid,
            tc.tile_pool(name="sbuf", bufs=3) as sbuf,
            tc.tile_pool(name="one", bufs=1) as onep,
            tc.tile_pool(name="gpool", bufs=2) as gpool,
            tc.tile_pool(name="s1pool", bufs=2) as s1pool,
            tc.tile_pool(name="idxp", bufs=2) as idxp,
            tc.tile_pool(name="psum", bufs=1, space="PSUM") as psum,
        ):
            # resident constants / state
            iota_t = resid.tile([P, P], dt.bfloat16)
            nc.sync.dma_start(iota_t[:], iota_in[:])
            ident_t = resid.tile([P, P], dt.float32)
            nc.sync.dma_start(ident_t[:], ident[:])
            s_t = resid.tile([P, OUT_DIM], dt.float32)
            nc.sync.dma_start(s_t[:], s_rep[:])
            dinvr = resid.tile([P, NBLK], dt.float32)
            nc.sync.dma_start(dinvr[:], dinv_r_t[:])
            dinvc = resid.tile([P, NBLK], dt.float32)
            nc.sync.dma_start(dinvc[:], dinv_c_t[:])
            wself = resid.tile([P, NBLK], dt.float32)
            nc.sync.dma_start(wself[:], wself_t[:])
            rel_t = resid.tile([P, n_mm], dt.bfloat16)
            nc.sync.dma_start(rel_t[:], rel_in[:])
            if nonsep:
                w_t = resid.tile([P, n_mm], dt.bfloat16)
                nc.sync.dma_start(w_t[:], w_in[:])

            w1_t = resid.tile([P, KB, HID_DIM], dt.bfloat16)
            nc.sync.dma_start(
                w1_t[:], W1b[:].rearrange("(kb p) m -> p kb m", p=P))
            w2_t = resid.tile([P, KB, OUT_DIM], dt.bfloat16)
            nc.sync.dma_start(
                w2_t[:], W2b[:].rearrange("(kb p) m -> p kb m", p=P))
            b1_t = resid.tile([P, MB], dt.float32)
            nc.sync.dma_start(b1_t[:], b1c[:])
            b2_t = resid.tile([OUT_DIM, 1], dt.float32)
            nc.sync.dma_start(b2_t[:], b2c[:])

            # state buffers [P, NBLK, OUT]
            h_prev = resid.tile([P, NBLK, OUT_DIM], dt.float32)
            y_cur = resid.tile([P, NBLK, OUT_DIM], dt.float32)
            out_acc = resid.tile([P, NBLK, OUT_DIM], dt.float32)
            nc.vector.memset(out_acc[:], 0.0)
            sk_t = resid.tile([P, NBLK], dt.float32)
            nc.vector.memset(y_cur[:], 0.0)

            # ---------------- dense front: h0 -> h_prev ----------------
            for rt in range(NRT_):
                xk = []
                for kb in range(KB):
                    t = sbuf.tile([P, NT], dt.bfloat16, tag="xk", bufs=8, name="xkt")
                    nc.sync.dma_start(
                        t[:], xT[kb * P:(kb + 1) * P, rt * NT:(rt + 1) * NT])
                    xk.append(t)
                h0 = []
                for mb in range(MB):
                    acc = psum.tile([P, NT], dt.float32, space="PSUM", tag="mm1", bufs=1)
                    for kb in range(KB):
                        nc.tensor.matmul(
                            out=acc[:],
                            lhsT=w1_t[:, kb, mb * P:(mb + 1) * P],
                            rhs=xk[kb][:],
                            start=(kb == 0), stop=(kb == KB - 1))
                    h0t = sbuf.tile([P, NT], dt.bfloat16, tag="h0", bufs=8)
                    nc.scalar.activation(
                        h0t[:], acc[:], mybir.ActivationFunctionType.Relu,
                        bias=b1_t[:, mb:mb + 1], scale=1.0)
                    h0.append(h0t)
                acc2 = psum.tile([P, NT], dt.float32, space="PSUM", tag="mm2", bufs=1)
                for kb in range(KB):
                    nc.tensor.matmul(
                        out=acc2[:OUT_DIM, :],
                        lhsT=w2_t[:, kb, :],
                        rhs=h0[kb][:],
                        start=(kb == 0), stop=(kb == KB - 1))
                hT = sbuf.tile([OUT_DIM, NT], dt.float32, tag="hT")
                nc.vector.tensor_scalar(
                    out=hT[:], in0=acc2[:OUT_DIM, :],
                    scalar1=b2_t[:, 0:1], scalar2=None,
                    op0=mybir.AluOpType.add)
                # transpose [64, NT] -> row layout [P, 64] x (NT/P)
                for st in range(NT // P):
                    pt = psum.tile([P, OUT_DIM], dt.float32, space="PSUM",
                                   tag="tr", bufs=2)
                    nc.tensor.transpose(
                        out=pt[:, :],
                        in_=hT[:, st * P:(st + 1) * P],
                        identity=ident_t[:OUT_DIM, :OUT_DIM])
                    blk = rt * (NT // P) + st
                    nc.vector.tensor_copy(h_prev[:, blk, :], pt[:, :])

            # ---------------- hops ----------------
            for hop in range(n_hops):
                # epilogue of previous state: S_k & out_acc for h_prev
                # S_k = sigmoid(sum_f h_prev*s) ; out_acc += S_k * h_prev
                tmp = onep.tile([P, NBLK, OUT_DIM], dt.float32, tag="tmp")
                nc.vector.tensor_tensor(
                    out=tmp[:], in0=h_prev[:],
                    in1=s_t[:].unsqueeze(1).to_broadcast([P, NBLK, OUT_DIM]),
                    op=mybir.AluOpType.mult)
                nc.vector.tensor_reduce(
                    out=sk_t[:], in_=tmp[:], axis=mybir.AxisListType.X,
                    op=mybir.AluOpType.add)
                nc.scalar.activation(
                    sk_t[:], sk_t[:], mybir.ActivationFunctionType.Sigmoid)
                nc.vector.tensor_tensor(
                    out=tmp[:], in0=h_prev[:],
                    in1=sk_t[:].unsqueeze(2).to_broadcast([P, NBLK, OUT_DIM]),
                    op=mybir.AluOpType.mult)
                nc.vector.tensor_add(out_acc[:], out_acc[:], tmp[:])

                # h_dup staging: hd = dinv_c * h_prev, cast bf16, dup halves
                hd = onep.tile([P, NBLK, OUT_DIM], dt.bfloat16, tag="hd")
                nc.vector.tensor_tensor(
                    out=hd[:], in0=h_prev[:],
                    in1=dinvc[:].unsqueeze(2).to_broadcast([P, NBLK, OUT_DIM]),
                    op=mybir.AluOpType.mult)
                # write both 64-halves of cc_in rows
                cc_v = cc_in[:].rearrange("(b p) f -> p b f", p=P)
                nc.sync.dma_start(cc_v[:, :, 0:OUT_DIM], hd[:])
                nc.sync.dma_start(cc_v[:, :, OUT_DIM:2 * OUT_DIM], hd[:])

                for q in range(4 if not skip_ag else 0):
                    nc.gpsimd.collective_compute(
                        "AllGather", mybir.AluOpType.bypass,
                        ins=[cc_in[q * QUART:(q + 1) * QUART, :]],
                        outs=[cc_out[q * GQ:(q + 1) * GQ, :]],
                        replica_groups=rg)

                # gather + S1 + PE reduce
                open_blk = {}
                ci = 0  # global chunk index
                for (w, start, n_e) in calls:
                    ncall = n_e // P
                    idxt = idxp.tile([P, GCH // 16], dt.int16, tag="idx")
                    nc.sync.dma_start(
                        idxt[:, :n_e // 16],
                        idx16[:, start // 16:(start + n_e) // 16])
                    g = gpool.tile([P, GCH // P, P], dt.bfloat16, tag="g")
                    if skip_gather:
                        nc.vector.memset(g[:, :ncall, :], 0.5)
                    else:
                        nc.gpsimd.dma_gather(
                            out_ap=g[:, :ncall, :],
                            in_ap=cc_out[w * GQ:(w + 1) * GQ, :],
                            idxs_ap=idxt[:, :n_e // 16],
                            num_idxs=n_e,
                            num_idxs_reg=n_e,
                            elem_size=P,
                            single_packet=os.environ.get("K_SP", "0") == "1",
                            queue_num=ci % int(os.environ.get("K_NQ", str(NQUEUES))),
                        )
                    s1 = s1pool.tile([P, GCH // P, P], dt.bfloat16, tag="s1")
                    nc.vector.tensor_tensor(
                        out=s1[:, :ncall, :],
                        in0=iota_t[:].unsqueeze(1).to_broadcast([P, ncall, P]),
                        in1=rel_t[:, ci:ci + ncall].unsqueeze(2)
                            .to_broadcast([P, ncall, P]),
                        op=mybir.AluOpType.is_equal)
                    if nonsep:
                        nc.vector.tensor_tensor(
                            out=s1[:, :ncall, :], in0=s1[:, :ncall, :],
                            in1=w_t[:, ci:ci + ncall].unsqueeze(2)
                                .to_broadcast([P, ncall, P]),
                            op=mybir.AluOpType.mult)
                    for j in range(ncall):
                        t = ci + j
                        b = int(chunk_blk[t])
                        wv = int(chunk_win[t])
                        key = (wv, b)
                        if key not in open_blk:
                            open_blk[key] = psum.tile(
                                [P, OUT_DIM], dt.float32, space="PSUM",
                                tag="acc", name="accb", bufs=4)
                        first = (t == 0 or int(chunk_blk[t - 1]) != b
                                 or int(chunk_win[t - 1]) != wv)
                        last = (t == n_mm - 1 or int(chunk_blk[t + 1]) != b
                                or int(chunk_win[t + 1]) != wv)
                        nc.tensor.matmul(
                            out=open_blk[key][:],
                            lhsT=s1[:, j, :],
                            rhs=g[:, j, 0:OUT_DIM],
                            start=first, stop=last)
                        if last:
                            pt = open_blk.pop(key)
                            if wv == first_win.get(b, -1):
                                nc.vector.tensor_copy(y_cur[:, b, :], pt[:])
                            else:
                                nc.vector.tensor_add(
                                    y_cur[:, b, :], y_cur[:, b, :], pt[:])
                    ci += ncall

                # finalize y: y = dinv_r*y + wself*h_prev
                nc.vector.tensor_tensor(
                    out=y_cur[:], in0=y_cur[:],
                    in1=dinvr[:].unsqueeze(2).to_broadcast([P, NBLK, OUT_DIM]),
                    op=mybir.AluOpType.mult)
                tmp2 = onep.tile([P, NBLK, OUT_DIM], dt.float32, tag="tmp")
                nc.vector.tensor_tensor(
                    out=tmp2[:], in0=h_prev[:],
                    in1=wself[:].unsqueeze(2).to_broadcast([P, NBLK, OUT_DIM]),
                    op=mybir.AluOpType.mult)
                nc.vector.tensor_add(y_cur[:], y_cur[:], tmp2[:])
                # swap: h_prev <- y_cur
                nc.vector.tensor_copy(h_prev[:], y_cur[:])

            # final hop's S_k/out_acc
            tmp = onep.tile([P, NBLK, OUT_DIM], dt.float32, tag="tmp")
            nc.vector.tensor_tensor(
                out=tmp[:], in0=h_prev[:],
                in1=s_t[:].unsqueeze(1).to_broadcast([P, NBLK, OUT_DIM]),
                op=mybir.AluOpType.mult)
            nc.vector.tensor_reduce(
                out=sk_t[:], in_=tmp[:], axis=mybir.AxisListType.X,
                op=mybir.AluOpType.add)
            nc.scalar.activation(
                sk_t[:], sk_t[:], mybir.ActivationFunctionType.Sigmoid)
            nc.vector.tensor_tensor(
                out=tmp[:], in0=h_prev[:],
                in1=sk_t[:].unsqueeze(2).to_broadcast([P, NBLK, OUT_DIM]),
                op=mybir.AluOpType.mult)
            nc.vector.tensor_add(out_acc[:], out_acc[:], tmp[:])

            nc.sync.dma_start(
                out_d[:].rearrange("(b p) f -> p b f", p=P), out_acc[:])

    nc.compile()
    return nc


class _Runner:
    """Compile once via bass2jax/PJRT, execute on demand (timeable)."""

    def __init__(self, nc, in_maps, n_cores=NCORES):
        import jax
        from jax.experimental.shard_map import shard_map
        from jax.sharding import Mesh, PartitionSpec
        import concourse.mybir as mybir
        from concourse import bass2jax

        bass2jax.install_neuronx_cc_hook()
        self._jax = jax
        self.nc = nc
        partition_name = (
            nc.partition_id_tensor.name if nc.partition_id_tensor else None)
        in_names, out_names, out_avals, zero_outs = [], [], [], []
        for alloc in nc.m.functions[0].allocations:
            if not isinstance(alloc, mybir.MemoryLocationSet):
                continue
            name = alloc.memorylocations[0].name
            if alloc.kind == "ExternalInput":
                if name != partition_name and name != (
                        nc.dbg_addr.name if nc.dbg_addr else None):
                    in_names.append(name)
            elif alloc.kind == "ExternalOutput":
                shape = tuple(alloc.tensor_shape)
                dtype = mybir.dt.np(alloc.dtype)
                out_names.append(name)
                out_avals.append(jax.core.ShapedArray(shape, dtype))
                zero_outs.append(np.zeros(shape, dtype))
        n_params = len(in_names)
        n_outs = len(out_avals)
        all_in_names = list(in_names) + list(out_names)
        if nc.dbg_addr is not None:
            all_in_names.append(nc.dbg_addr.name)
        if partition_name is not None:
            all_in_names.append(partition_name)
        donate = tuple(range(n_params, n_params + n_outs))

        def _body(*args):
            operands = list(args)
            if nc.dbg_addr is not None:
                operands.append(jax.numpy.zeros((1, 2), jax.numpy.uint32))
            if partition_name is not None:
                operands.append(bass2jax.partition_id_tensor())
            outs = bass2jax._bass_exec_p.bind(
                *operands,
                out_avals=tuple(out_avals),
                in_names=tuple(all_in_names),
                out_names=tuple(out_names),
                lowering_input_output_aliases=(),
                sim_require_finite=False,
                sim_require_nnan=False,
                nc=nc,
            )
            return tuple(outs)

        devices = jax.devices()[:n_cores]
        mesh = Mesh(np.asarray(devices), ("core",))
        from jax.sharding import NamedSharding
        shard = NamedSharding(mesh, PartitionSpec("core"))
        self._fn = jax.jit(
            shard_map(_body, mesh=mesh,
                      in_specs=(PartitionSpec("core"),) * (n_params + n_outs),
                      out_specs=(PartitionSpec("core"),) * len(out_names),
                      check_rep=False),
            keep_unused=True)
        concat_in = [
            np.concatenate([np.asarray(in_maps[c][k]) for c in range(n_cores)],
                           axis=0)
            for k in in_names
        ]
        self._dev_in = [jax.device_put(a, shard) for a in concat_in]
        jax.block_until_ready(self._dev_in)
        self._zero_outs = [
            jax.device_put(
                np.zeros((n_cores * z.shape[0], *z.shape[1:]), z.dtype), shard)
            for z in zero_outs
        ]
        jax.block_until_ready(self._zero_outs)
        self.out_names = out_names
        self.out_avals = out_avals
        self.n_cores = n_cores

    def run_once(self):
        outs = self._fn(*self._dev_in, *self._zero_outs)
        self._jax.block_until_ready(outs)
        return outs

    def results(self, outs):
        return [
            {name: np.asarray(outs[i]).reshape(
                self.n_cores, *self.out_avals[i].shape)[c]
             for i, name in enumerate(self.out_names)}
            for c in range(self.n_cores)
        ]

    def time(self, iters=5, warmup=1):
        import time as _time
        for _ in range(warmup):
            self.run_once()
        ts = []
        for _ in range(iters):
            t0 = _time.perf_counter()
            self.run_once()
            ts.append(_time.perf_counter() - t0)
        return min(ts), None


LAST_RUNNER = None


def kernel(x, row, col, edge_w, W1, b1, W2, b2, s):
    global LAST_RUNNER
    plan, idx_stream, rel_stream, w_stream, vecs = _build_plan(row, col, edge_w)
    nonsep = not plan["sep"]

    nc = _build_program(plan, nonsep)

    idx16 = _pack_idx16(idx_stream)
    xt = _perm_x(x)
    n_mm = plan["n_mm"]

    iota = np.broadcast_to(
        np.arange(P, dtype=np.float32), (P, P)).astype(_bf16)
    ident = np.eye(P, dtype=np.float32)
    s_rep = np.broadcast_to(
        np.asarray(s, np.float32).reshape(1, OUT_DIM), (P, OUT_DIM)).copy()
    W1b = np.asarray(W1, np.float32).astype(_bf16)
    W2b = np.asarray(W2, np.float32).astype(_bf16)
    b1c = np.asarray(b1, np.float32).reshape(HID_DIM // P, P).T.copy()
    b2c = np.asarray(b2, np.float32).reshape(OUT_DIM, 1)

    in_maps = []
    for c in range(NCORES):
        m = {
            "xT": xt[c],
            "W1b": W1b, "W2b": W2b, "b1c": b1c, "b2c": b2c,
            "ident": ident, "iota_in": iota, "s_rep": s_rep,
            "dinv_r_t": vecs["dinv_r"][c], "dinv_c_t": vecs["dinv_c"][c],
            "wself_t": vecs["wself"][c],
            "idx16": idx16[c],
            "rel_in": rel_stream[c].reshape(n_mm, P).T.astype(_bf16),
        }
        if nonsep:
            m["w_in"] = w_stream[c].reshape(n_mm, P).T.astype(_bf16)
        in_maps.append(m)

    runner = _Runner(nc, in_maps, NCORES)
    LAST_RUNNER = runner
    outs = runner.run_once()
    res = [m["out"] for m in runner.results(outs)]
    return _unperm_out(res).astype(np.float32)

